# revision 26
# baseline (speedup 1.0000x reference)
"""Trainium2 Bass kernel for nn_MultiHeadedAttention_51737176047655.

Multi-head attention with Music-Transformer relative position bias
(skew trick), B=4, L=1024, D=1024, 16 heads, head_dim=64.

Sharding (8 cores): core = 2*b + hg  -> batch b in [0,4), head-group hg in
[0,2).  Each core computes 8 heads for one batch over the full sequence:
  - Wq/Wk/Wv column-sharded [1024, 512], Wo row-sharded [512, 1024]
  - per-core output is a partial [1024, 1024]; host sums the two
    head-group partials per batch (standard TP unshard) and adds bo.

Device algorithm per core (matmuls bf16 in / f32 PSUM accumulate):
  qT/kT/vT arrive host-transposed [d, l]; projections give qhT/khT
  [d', l] (transposed) and vh [l, d'] (natural, with a ones column per
  head for softmax sums).  QE = qh e^T is computed per head (only the
  m < l0+128 span that survives the tri mask), masked via a shifted-tri
  "slab" multiply, and written to a DRAM scratch in the padded layout
  (row stride 1025); reading rows back with stride 1024 materializes
  the skewed Srel exactly (the reference's pad+reshape trick).  scores
  are computed TRANSPOSED (scores^T = kh qh^T, head pairs packed into
  PE row groups via tile_position) and Srel^T is accumulated into the
  same PSUM bank by transpose-by-identity matmuls, skipping
  statically-zero 128x128 blocks; exp via ScalarE (scale=1/8) ->
  unnormalized attn^T (bf16); ctx^T_aug = [vh|1]^T attn^T per head
  (row 64 = softmax denominators Z); 1/Z via a single ScalarE
  Reciprocal activation straight off the PSUM Z row, broadcast across
  64 partitions with a K=1 PE matmul, and applied by the DVE while
  packing ctx^T head-pairs; out = ctx Wo, emitted bf16 (host
  accumulates in f32).
  The TensorE instruction stream is interleaved at j-tile granularity
  (scores of head h / attnV of head h-1 / QE of head h+2, and attnV of
  the last head with the first half of the output projection) so the
  in-order PE never stalls on the exp/normalize pipelines; this keeps
  the PE clock at its top p-state.  DMA count is minimized: batched
  input loads, one sliding-window batched read for the low-half Srel
  stripes, persistent pre-zeroed stripe staging tiles.
No max-subtraction in softmax: logits are ~N(0, 1.4^2), far inside
fp32/exp range (validated vs reference at ~1e-6 in fp32 emulation).
"""

import math
import sys

import numpy as np

sys.path.insert(0, "/opt/trn_rl_repo")

import ml_dtypes  # noqa: E402

BF16 = ml_dtypes.bfloat16

# Problem constants (hardcoded per contract)
B = 4
L = 1024
D = 1024
H = 16
HD = 64
H_LOC = 8  # heads per core
DG = 512  # d' columns per core (H_LOC * HD)
NCORES = 8
MAX_SEQ = 2048
PAD = L + 1  # 1025, padded row stride of the skew scratch
FLAT = L * PAD  # 1049600 elements per head scratch

NLT = L // 128  # 8 l-tiles
NDT = D // 128  # 8 contraction tiles
NPAIR = H_LOC // 2  # 4 head pairs


def _build_bass():
    """Build the single-core SPMD Bass program (same program, per-core data)."""
    import concourse.bass as bass
    import concourse.tile as tile
    from concourse import bacc, mybir

    f32 = mybir.dt.float32
    bf16 = mybir.dt.bfloat16
    Exp = mybir.ActivationFunctionType.Exp
    mult = mybir.AluOpType.mult

    nc = bacc.Bacc(
        "TRN2", target_bir_lowering=False, debug=False, enable_asserts=False
    )

    # ---- kernel I/O (qT/kT/vT are host-transposed [d, l]) ----
    qT_d = nc.declare_dram_parameter("qT", [D, L], bf16, isOutput=False)
    kT_d = nc.declare_dram_parameter("kT", [D, L], bf16, isOutput=False)
    vT_d = nc.declare_dram_parameter("vT", [D, L], bf16, isOutput=False)
    wq_d = nc.declare_dram_parameter("wq", [D, DG], bf16, isOutput=False)
    wk_d = nc.declare_dram_parameter("wk", [D, DG], bf16, isOutput=False)
    wv_d = nc.declare_dram_parameter("wv", [D, DG], bf16, isOutput=False)
    wo_d = nc.declare_dram_parameter("wo", [DG, D], bf16, isOutput=False)
    e2_d = nc.declare_dram_parameter("e2", [128, L], bf16, isOutput=False)
    slab_d = nc.declare_dram_parameter("slab", [128, 640], bf16, isOutput=False)
    out_d = nc.declare_dram_parameter("out", [L, D], bf16, isOutput=True)

    # skew scratch, one padded buffer per local head
    scratch = [nc.dram_tensor(f"skew{h}", [FLAT], bf16) for h in range(H_LOC)]

    # block (lt, jt) of Srel is identically zero unless piece A
    # (j <= 2l-1023) or piece B (l+2 <= j <= 2l+3) intersects it.
    def srel_block_nonzero(lt, jt):
        l1 = 128 * lt + 127
        j0, j1 = 128 * jt, 128 * jt + 127
        a = 2 * l1 - 1023 >= j0
        b = (j1 >= 128 * lt + 2) and (j0 <= 2 * l1 + 3)
        return a or b

    with tile.TileContext(nc) as tc:
        from contextlib import ExitStack

        with ExitStack() as outer:
            # ---------------- persistent pools ----------------
            persist = outer.enter_context(tc.tile_pool(name="persist", bufs=1))
            # projection outputs (live through whole kernel)
            qhT = persist.tile([128, NPAIR, L], bf16)  # [part, pair, l]
            khT = persist.tile([128, NPAIR, L], bf16)
            # vh with ones column per head: [part(j%128), jt, head, 65]
            vh = persist.tile([128, NLT, H_LOC, HD + 1], bf16)
            e2_sb = persist.tile([128, L], bf16)
            slab_sb = persist.tile([128, 640], bf16)
            ctxp = persist.tile([128, NPAIR, L], bf16)  # packed ctx^T per pair
            # all-ones stationary for the K=1 1/Z PE broadcast
            ones1 = persist.tile([1, 64], bf16, name="ones1")
            # persistent stripe staging tiles, pre-zeroed once; data spans are
            # rewritten per head, zero-col/tail spans stay zero across reuse
            stripes = [
                [persist.tile([128, 4, PAD], bf16, name=f"st{lh}{par}")
                 for par in range(2)]
                for lh in range(2)
            ]

            nc.sync.dma_start(out=e2_sb, in_=e2_d[:, :])
            nc.sync.dma_start(out=slab_sb, in_=slab_d[:, :])
            nc.vector.memset(vh[:, :, :, HD : HD + 1], 1.0)
            nc.vector.memset(ones1, 1.0)
            for lh in range(2):
                for par in range(2):
                    nc.gpsimd.memset(stripes[lh][par], 0.0)

            # ---------------- phase 1+2: loads + projections ----
            with ExitStack() as outer2:
                sc_ps = outer2.enter_context(
                    tc.tile_pool(name="sc_ps", bufs=5, space="PSUM")
                )
                qe_ps = sc_ps  # QE shares the scores PSUM slots (tag "sc")
                ctx_ps = None  # opened after mm_ps closes (PSUM bank budget)
                attT = outer2.enter_context(tc.tile_pool(name="attT", bufs=4))
                srl = outer2.enter_context(tc.tile_pool(name="srl", bufs=2))
                zp = outer2.enter_context(tc.tile_pool(name="zp", bufs=2))

                ident = persist.tile([128, 128], bf16, name="ident")
                from concourse.masks import make_identity

                make_identity(nc, ident)

                # short-lived input pools opened last (LIFO close order)
                tin_blk = ExitStack()
                tin = tin_blk.enter_context(tc.tile_pool(name="tin", bufs=1))
                mm_ps = tin_blk.enter_context(
                    tc.tile_pool(name="mm_ps", bufs=2, space="PSUM")
                )

                qT = tin.tile([128, NDT, L], bf16, name="qT")
                kT = tin.tile([128, NDT, L], bf16, name="kT")
                vT = tin.tile([128, NDT, L], bf16, name="vT")
                wq_sb = tin.tile([128, NDT, DG], bf16, name="wq")
                wk_sb = tin.tile([128, NDT, DG], bf16, name="wk")
                wv_sb = tin.tile([128, NDT, DG], bf16, name="wv")

                def load_xT(dst, src_d):
                    src = bass.AP(
                        tensor=src_d,
                        offset=0,
                        ap=[[L, 128], [128 * L, NDT], [1, L]],
                    )
                    nc.sync.dma_start(out=dst, in_=src)

                def load_w(dst, src_d):
                    src = bass.AP(
                        tensor=src_d,
                        offset=0,
                        ap=[[DG, 128], [128 * DG, NDT], [1, DG]],
                    )
                    nc.sync.dma_start(out=dst, in_=src)

                # q + Wq first so qh projections (and QE) can start early;
                # whole-tensor batched loads (one trigger per tensor)
                load_w(wq_sb, wq_d)
                load_xT(qT, qT_d)
                load_w(wk_sb, wk_d)
                load_xT(kT, kT_d)
                load_w(wv_sb, wv_d)
                load_xT(vT, vT_d)

                def proj_pair(w_sb, xT, dst, p):
                    for lh in range(2):
                        ps = mm_ps.tile([128, 512], f32, name="proj_ps", tag="mm")
                        lsl = slice(512 * lh, 512 * (lh + 1))
                        for dt in range(NDT):
                            nc.tensor.matmul(
                                ps,
                                w_sb[:, dt, 128 * p : 128 * (p + 1)],
                                xT[:, dt, lsl],
                                start=(dt == 0),
                                stop=(dt == NDT - 1),
                            )
                        nc.scalar.copy(dst[:, p, lsl], ps)

                def vh_tile(jt):
                    ps = mm_ps.tile([128, 512], f32, name="vh_ps", tag="mm")
                    jsl = slice(128 * jt, 128 * (jt + 1))
                    for dt in range(NDT):
                        nc.tensor.matmul(
                            ps,
                            vT[:, dt, jsl],
                            wv_sb[:, dt, :],
                            start=(dt == 0),
                            stop=(dt == NDT - 1),
                        )
                    # scatter 512 d' columns into per-head [64] slots with one
                    # strided copy (dst strides over the 65-wide head slots)
                    pv = ps[0:128, :]
                    ps3 = bass.AP(
                        tensor=pv.tensor,
                        offset=pv.offset,
                        ap=[list(pv.ap)[0], [HD, H_LOC], [1, HD]],
                    )
                    nc.scalar.copy(vh[:, jt, :, 0:HD], ps3)

                def qe_lt(h, lt):
                    """QE + masked padded stripe row-block lt for head h;
                    after the 4th block of an l-half, emit the stripe DMA."""
                    p, hl = divmod(h, 2)
                    rows = slice(64 * hl, 64 * (hl + 1))
                    tp = (64 * hl, 0)
                    lh, a = divmod(lt, 4)
                    big = stripes[lh][h % 2]
                    l0 = 128 * lt
                    lsl = slice(l0, l0 + 128)
                    stripe = big[:, a, :]
                    # QE only over the m-range that survives the tri mask
                    if lt <= 3:
                        n0 = l0 + 128
                        psm = qe_ps.tile([128, 512], f32, name="qe", tag="sc")
                        nc.tensor.matmul(
                            psm[:, 0:n0],
                            qhT[rows, p, lsl],
                            e2_sb[rows, 0:n0],
                            start=True,
                            stop=True,
                            tile_position=tp,
                        )
                        nc.vector.tensor_tensor(
                            stripe[:, 1 : 1 + n0],
                            psm[:, 0:n0],
                            slab_sb[:, 512 - l0 : 640],
                            mult,
                        )
                    else:
                        psm = qe_ps.tile([128, 512], f32, name="qe", tag="sc")
                        nc.tensor.matmul(
                            psm,
                            qhT[rows, p, lsl],
                            e2_sb[rows, 0:512],
                            start=True,
                            stop=True,
                            tile_position=tp,
                        )
                        n1 = l0 + 128 - 512
                        psm2 = qe_ps.tile([128, 512], f32, name="qe", tag="sc")
                        nc.tensor.matmul(
                            psm2[:, 0:n1],
                            qhT[rows, p, lsl],
                            e2_sb[rows, 512 : 512 + n1],
                            start=True,
                            stop=True,
                            tile_position=tp,
                        )
                        if lt == 4:
                            nc.vector.tensor_tensor(
                                stripe[:, 1:513],
                                psm,
                                slab_sb[:, 0:512],
                                mult,
                            )
                        else:
                            # m < 512 is fully below the diagonal: copy
                            nc.vector.tensor_copy(stripe[:, 1:513], psm)
                        nc.vector.tensor_tensor(
                            stripe[:, 513 : 1 + l0 + 128],
                            psm2[:, 0:n1],
                            slab_sb[:, 1024 - l0 : 640],
                            mult,
                        )
                    if a == 3:
                        # one DMA for the 4 padded stripes of this l-half
                        dst = bass.AP(
                            tensor=scratch[h],
                            offset=512 * lh * PAD,
                            ap=[[PAD, 128], [128 * PAD, 4], [1, PAD]],
                        )
                        nc.sync.dma_start(out=dst, in_=big)

                def srel_load(h, lh):
                    if lh == 0:
                        # low l-half: sliding 640-wide j-window per lt
                        # (window start 128*lt covers all nonzero blocks)
                        srel = srl.tile([128, 4, 640], bf16, name="srel")
                        src = bass.AP(
                            tensor=scratch[h],
                            offset=L,
                            ap=[[L, 128], [128 * L + 128, 4], [1, 640]],
                        )
                    else:
                        # high l-half: dense
                        srel = srl.tile([128, 4, L], bf16, name="srel")
                        src = bass.AP(
                            tensor=scratch[h],
                            offset=(512 * lh + 1) * L,
                            ap=[[L, 128], [128 * L, 4], [1, L]],
                        )
                    nc.sync.dma_start(out=srel, in_=src)
                    return srel

                def scores_tile(h, lh, jt, srel, att):
                    """scores^T + Srel^T + exp for one (l-half, j-tile)."""
                    p, hl = divmod(h, 2)
                    rows = slice(64 * hl, 64 * (hl + 1))
                    tp = (64 * hl, 0)
                    lsl = slice(512 * lh, 512 * (lh + 1))
                    jsl = slice(128 * jt, 128 * (jt + 1))
                    ps = sc_ps.tile([128, 512], f32, name="sc", tag="sc")
                    nzs = [
                        a for a in range(4)
                        if srel_block_nonzero(4 * lh + a, jt)
                    ]
                    # scores^T = kh qh^T for this (j-tile, l-half)
                    nc.tensor.matmul(
                        ps,
                        khT[rows, p, jsl],
                        qhT[rows, p, lsl],
                        start=True,
                        stop=(len(nzs) == 0),
                        tile_position=tp,
                    )
                    # += Srel^T via PE transpose-by-identity
                    for i, a in enumerate(nzs):
                        if lh == 0:
                            jr = slice(128 * (jt - a), 128 * (jt - a) + 128)
                            chunk = srel[:, a, jr]
                        else:
                            chunk = srel[:, a, jsl]
                        nc.tensor.matmul(
                            ps[:, 128 * a : 128 * a + 128],
                            chunk,
                            ident,
                            start=False,
                            stop=(i == len(nzs) - 1),
                        )
                    nc.scalar.activation(att[:, jt, :], ps, Exp, scale=0.125)

                def attnv_part(h, halves, lh, jt, cps):
                    nc.tensor.matmul(
                        cps[0 : HD + 1, :],
                        vh[:, jt, h, :],
                        halves[lh][:, jt, :],
                        start=(jt == 0),
                        stop=(jt == NLT - 1),
                    )

                def attnv_finish_a(cps0, zpack):
                    # stage the lh=0 Z row; the real finish happens in _b
                    nc.vector.tensor_copy(zpack[0:1, :], cps0[HD : HD + 1, :])

                def attnv_finish_b(h, cps_both, zpack):
                    p, hl = divmod(h, 2)
                    rows = slice(64 * hl, 64 * (hl + 1))
                    nc.vector.tensor_copy(
                        zpack[32:33, :], cps_both[1][HD : HD + 1, :]
                    )
                    # one DVE reciprocal covers both Z rows (0 and 32; the
                    # rows between hold garbage and are never read)
                    zinv = zp.tile([33, 512], f32, name="zinv")
                    nc.vector.reciprocal(zinv, zpack)
                    zinvb = [
                        zp.tile([1, 512], bf16, name="zinvb") for _ in range(2)
                    ]
                    for lh in range(2):
                        nc.vector.tensor_copy(
                            zinvb[lh], zinv[32 * lh : 32 * lh + 1, :]
                        )
                    for lh in range(2):
                        # broadcast across 64 partitions with a K=1 matmul
                        zb = sc_ps.tile([64, 512], f32, name="zb", tag="sc")
                        nc.tensor.matmul(
                            zb,
                            ones1,
                            zinvb[lh],
                            start=True,
                            stop=True,
                        )
                        zbs = zp.tile([64, 512], bf16, name="zbs")
                        nc.vector.tensor_copy(zbs, zb)
                        # normalize + pack into head-pair ctx^T (bf16)
                        nc.vector.tensor_tensor(
                            ctxp[rows, p, 512 * lh : 512 * (lh + 1)],
                            cps_both[lh][0:HD, :],
                            zbs,
                            mult,
                        )

                # ---- emission: projections first ----
                for p in range(NPAIR):
                    proj_pair(wq_sb, qT, qhT, p)
                for lt in range(8):
                    qe_lt(0, lt)
                for lt in range(8):
                    qe_lt(1, lt)
                for p in range(NPAIR):
                    proj_pair(wk_sb, kT, khT, p)
                for jt in range(NLT):
                    vh_tile(jt)
                tin_blk.close()
                ctx_ps = outer2.enter_context(
                    tc.tile_pool(name="ctx_ps", bufs=3, space="PSUM")
                )
                # wo lives in the space vacated by the input tiles; loaded
                # here (well before the output projection)
                wop = outer2.enter_context(tc.tile_pool(name="wop", bufs=1))
                wo_sb = wop.tile([128, NPAIR, D], bf16, name="wo")
                wo_src = bass.AP(
                    tensor=wo_d, offset=0, ap=[[D, 128], [128 * D, NPAIR], [1, D]]
                )
                nc.sync.dma_start(out=wo_sb, in_=wo_src)
                ost = outer2.enter_context(tc.tile_pool(name="ost", bufs=4))

                # ---- main pipeline: scores(h) / attnV(h-1) / QE(h+2)
                # interleaved at j-tile granularity so the in-order PE
                # always has a ready instruction ----
                pend = None
                for h in range(H_LOC):
                    srel0 = srel_load(h, 0)
                    att0 = attT.tile([128, NLT, 512], bf16, name="attnT")
                    att1 = attT.tile([128, NLT, 512], bf16, name="attnT")
                    cps_prev = {}
                    zpack = zp.tile([33, 512], f32, name="zpack")
                    for jt in range(NLT):
                        scores_tile(h, 0, jt, srel0, att0)
                        if pend is not None:
                            hp, halves = pend
                            if jt == 0:
                                cps_prev[0] = ctx_ps.tile(
                                    [128, 512], f32, name="cps", tag="cps"
                                )
                            attnv_part(hp, halves, 0, jt, cps_prev[0])
                        if jt % 2 == 1 and h + 2 < H_LOC:
                            qe_lt(h + 2, jt // 2)
                    if pend is not None:
                        attnv_finish_a(cps_prev[0], zpack)
                    srel1 = srel_load(h, 1)
                    for jt in range(NLT):
                        scores_tile(h, 1, jt, srel1, att1)
                        if pend is not None:
                            if jt == 0:
                                cps_prev[1] = ctx_ps.tile(
                                    [128, 512], f32, name="cps", tag="cps"
                                )
                            attnv_part(hp, halves, 1, jt, cps_prev[1])
                        if jt % 2 == 1 and h + 2 < H_LOC:
                            qe_lt(h + 2, 4 + jt // 2)
                    if pend is not None:
                        attnv_finish_b(hp, cps_prev, zpack)
                    pend = (h, [att0, att1])

                # ---- tail: attnV of the last head, interleaved with the
                # first half of the output projection ----
                def outproj_unit(lt, jh, o):
                    lsl = slice(128 * lt, 128 * (lt + 1))
                    jsl = slice(512 * jh, 512 * (jh + 1))
                    ps = sc_ps.tile([128, 512], f32, name="op", tag="sc")
                    for p in range(NPAIR):
                        nc.tensor.matmul(
                            ps,
                            ctxp[:, p, lsl],
                            wo_sb[:, p, jsl],
                            start=(p == 0),
                            stop=(p == NPAIR - 1),
                        )
                    nc.scalar.copy(o[:, jsl], ps)

                hp, halves = pend
                o_tiles = {}
                zpack = zp.tile([33, 512], f32, name="zpack")
                cps_last = {}
                cps_last[0] = ctx_ps.tile([128, 512], f32, name="cps", tag="cps")
                for jt in range(NLT):
                    attnv_part(hp, halves, 0, jt, cps_last[0])
                attnv_finish_a(cps_last[0], zpack)
                cps_last[1] = ctx_ps.tile([128, 512], f32, name="cps", tag="cps")
                for jt in range(NLT):
                    attnv_part(hp, halves, 1, jt, cps_last[1])
                attnv_finish_b(hp, cps_last, zpack)
                # out-proj over the lh=0 l-tiles follows the last normalize
                for jt in range(NLT):
                    lt, jh = jt // 2, jt % 2
                    if jh == 0:
                        o_tiles[lt] = ost.tile([128, D], bf16, name="o")
                    outproj_unit(lt, jh, o_tiles[lt])
                    if jh == 1:
                        lsl = slice(128 * lt, 128 * (lt + 1))
                        nc.sync.dma_start(out=out_d[lsl, :], in_=o_tiles[lt])
                for lt in range(4, NLT):
                    o = ost.tile([128, D], bf16, name="o")
                    for jh in range(2):
                        outproj_unit(lt, jh, o)
                    lsl = slice(128 * lt, 128 * (lt + 1))
                    nc.sync.dma_start(out=out_d[lsl, :], in_=o)

    nc.compile()
    return nc


TRACE = False
TRACE_KWARGS = {}
LAST_RESULT = None

_NC_CACHE = None


def _get_nc():
    global _NC_CACHE
    if _NC_CACHE is None:
        _NC_CACHE = _build_bass()
    return _NC_CACHE


def make_in_maps(k, v, q, E, Wk, Wv, Wq, Wo):
    """Host-side sharding: returns per-core input dicts."""
    eT = np.ascontiguousarray(E[MAX_SEQ - L :, :].T)  # [64, 1024]
    e2 = np.concatenate([eT, eT], axis=0).astype(BF16)  # [128, 1024]
    slab = (
        (np.arange(640)[None, :] - 512) <= np.arange(128)[:, None]
    ).astype(BF16)
    qkvT = {}
    for b in range(B):
        qkvT[b] = (
            np.ascontiguousarray(np.asarray(q[b]).T).astype(BF16),
            np.ascontiguousarray(np.asarray(k[b]).T).astype(BF16),
            np.ascontiguousarray(np.asarray(v[b]).T).astype(BF16),
        )
    in_maps = []
    for core in range(NCORES):
        b, hg = divmod(core, 2)
        csl = slice(DG * hg, DG * (hg + 1))
        qTb, kTb, vTb = qkvT[b]
        in_maps.append(
            {
                "qT": qTb,
                "kT": kTb,
                "vT": vTb,
                "wq": np.ascontiguousarray(Wq[:, csl]).astype(BF16),
                "wk": np.ascontiguousarray(Wk[:, csl]).astype(BF16),
                "wv": np.ascontiguousarray(Wv[:, csl]).astype(BF16),
                "wo": np.ascontiguousarray(Wo[DG * hg : DG * (hg + 1), :]).astype(BF16),
                "e2": e2,
                "slab": slab,
            }
        )
    return in_maps


def kernel(
    k,
    v,
    q,
    mask,
    E,
    Wk,
    bk,
    Wv,
    bv,
    Wq,
    bq,
    Wo,
    bo,
):
    k = np.asarray(k, np.float32)
    v = np.asarray(v, np.float32)
    q = np.asarray(q, np.float32)
    E = np.asarray(E, np.float32)
    Wk = np.asarray(Wk, np.float32)
    Wv = np.asarray(Wv, np.float32)
    Wq = np.asarray(Wq, np.float32)
    Wo = np.asarray(Wo, np.float32)
    mask = np.asarray(mask)
    assert bool(mask.all()), "kernel specialized for all-true mask"
    for bias in (bk, bv, bq):
        assert not np.any(np.asarray(bias)), "kernel specialized for zero qkv biases"
    bo = np.asarray(bo, np.float32)

    from concourse.bass_utils import run_bass_kernel_spmd

    nc = _get_nc()
    in_maps = make_in_maps(k, v, q, E, Wk, Wv, Wq, Wo)
    res = run_bass_kernel_spmd(
        nc, in_maps, core_ids=list(range(NCORES)), trace=TRACE, **TRACE_KWARGS
    )
    global LAST_RESULT
    LAST_RESULT = res
    out = np.zeros((B, L, D), np.float32)
    for core in range(NCORES):
        b = core // 2
        out[b] += np.asarray(res.results[core]["out"], np.float32)
    out += bo[None, None, :]
    return out


# revision 32
# speedup vs baseline: 1.3308x; 1.3308x over previous
"""Trainium2 Bass kernel for nn_MultiHeadedAttention_51737176047655.

Multi-head attention with Music-Transformer relative position bias
(skew trick), B=4, L=1024, D=1024, 16 heads, head_dim=64.

Sharding (8 cores): core = 2*b + hg  -> batch b in [0,4), head-group hg in
[0,2).  Each core computes 8 heads for one batch over the full sequence:
  - Wq/Wk/Wv column-sharded [1024, 512], Wo row-sharded [512, 1024]
  - per-core output is a partial [1024, 1024]; host sums the two
    head-group partials per batch (standard TP unshard) and adds bo.

Device algorithm per core (matmuls bf16 in / f32 PSUM accumulate):
  qT/kT/vT arrive host-transposed [d, l]; projections give qhT/khT
  [d', l] (transposed) and vh [l, d'] (natural, with a ones column per
  head for softmax sums).  QE = qh e^T is computed per head (only the
  m < l0+128 span that survives the tri mask), masked via a shifted-tri
  "slab" multiply, and written to a DRAM scratch in the padded layout
  (row stride 1025); reading rows back with stride 1024 materializes
  the skewed Srel exactly (the reference's pad+reshape trick).  scores
  are computed TRANSPOSED (scores^T = kh qh^T, head pairs packed into
  PE row groups via tile_position) and Srel^T is accumulated into the
  same PSUM bank by transpose-by-identity matmuls, skipping
  statically-zero 128x128 blocks; exp via ScalarE (scale=1/8) ->
  unnormalized attn^T (bf16); ctx^T_aug = [vh|1]^T attn^T per head
  (row 64 = softmax denominators Z); 1/Z via a single ScalarE
  Reciprocal activation straight off the PSUM Z row, broadcast across
  64 partitions with a K=1 PE matmul, and applied by the DVE while
  packing ctx^T head-pairs; out = ctx Wo, emitted bf16 (host
  accumulates in f32).
  The TensorE instruction stream is interleaved at j-tile granularity
  (scores of head h / attnV of head h-1 / QE of head h+2, and attnV of
  the last head with the first half of the output projection) so the
  in-order PE never stalls on the exp/normalize pipelines; this keeps
  the PE clock at its top p-state.  DMA count is minimized: batched
  input loads, one sliding-window batched read for the low-half Srel
  stripes, persistent pre-zeroed stripe staging tiles.
No max-subtraction in softmax: logits are ~N(0, 1.4^2), far inside
fp32/exp range (validated vs reference at ~1e-6 in fp32 emulation).
"""

import math
import sys

import numpy as np

sys.path.insert(0, "/opt/trn_rl_repo")

import ml_dtypes  # noqa: E402

BF16 = ml_dtypes.bfloat16

# Problem constants (hardcoded per contract)
B = 4
L = 1024
D = 1024
H = 16
HD = 64
H_LOC = 8  # heads per core
DG = 512  # d' columns per core (H_LOC * HD)
NCORES = 8
MAX_SEQ = 2048
PAD = L + 1  # 1025, padded row stride of the skew scratch
FLAT = L * PAD  # 1049600 elements per head scratch

NLT = L // 128  # 8 l-tiles
NDT = D // 128  # 8 contraction tiles
NPAIR = H_LOC // 2  # 4 head pairs


def _build_bass():
    """Build the single-core SPMD Bass program (same program, per-core data)."""
    import concourse.bass as bass
    import concourse.tile as tile
    from concourse import bacc, mybir

    f32 = mybir.dt.float32
    bf16 = mybir.dt.bfloat16
    Exp = mybir.ActivationFunctionType.Exp
    mult = mybir.AluOpType.mult

    nc = bacc.Bacc(
        "TRN2", target_bir_lowering=False, debug=False, enable_asserts=False
    )

    # ---- kernel I/O (qT/kT/vT are host-transposed [d, l]) ----
    qT_d = nc.declare_dram_parameter("qT", [D, L], bf16, isOutput=False)
    kT_d = nc.declare_dram_parameter("kT", [D, L], bf16, isOutput=False)
    vT_d = nc.declare_dram_parameter("vT", [D, L], bf16, isOutput=False)
    wq_d = nc.declare_dram_parameter("wq", [D, DG], bf16, isOutput=False)
    wk_d = nc.declare_dram_parameter("wk", [D, DG], bf16, isOutput=False)
    wv_d = nc.declare_dram_parameter("wv", [D, DG], bf16, isOutput=False)
    wo_d = nc.declare_dram_parameter("wo", [DG, D], bf16, isOutput=False)
    e2_d = nc.declare_dram_parameter("e2", [128, L], bf16, isOutput=False)
    slab_d = nc.declare_dram_parameter("slab", [128, 640], bf16, isOutput=False)
    out_d = nc.declare_dram_parameter("out", [L, D], bf16, isOutput=True)

    # skew scratch, one padded buffer per local head
    scratch = [nc.dram_tensor(f"skew{h}", [FLAT], bf16) for h in range(H_LOC)]

    # block (lt, jt) of Srel is identically zero unless piece A
    # (j <= 2l-1023) or piece B (l+2 <= j <= 2l+3) intersects it.
    def srel_block_nonzero(lt, jt):
        l1 = 128 * lt + 127
        j0, j1 = 128 * jt, 128 * jt + 127
        a = 2 * l1 - 1023 >= j0
        b = (j1 >= 128 * lt + 2) and (j0 <= 2 * l1 + 3)
        return a or b

    with tile.TileContext(nc) as tc:
        from contextlib import ExitStack

        with ExitStack() as outer:
            # ---------------- persistent pools ----------------
            persist = outer.enter_context(tc.tile_pool(name="persist", bufs=1))
            # projection outputs (live through whole kernel)
            qhT = persist.tile([128, NPAIR, L], bf16)  # [part, pair, l]
            khT = persist.tile([128, NPAIR, L], bf16)
            # vh with ones column per head: [part(j%128), jt, head, 65]
            vh = persist.tile([128, NLT, H_LOC, HD + 1], bf16)
            e2_sb = persist.tile([128, L], bf16)
            slab_sb = persist.tile([128, 640], bf16)
            ctxp = persist.tile([128, NPAIR, L], bf16)  # packed ctx^T per pair
            # all-ones stationary for the K=1 1/Z PE broadcast
            ones1 = persist.tile([1, 64], bf16, name="ones1")
            # persistent stripe staging tiles, pre-zeroed once; data spans are
            # rewritten per head, zero-col/tail spans stay zero across reuse
            stripes = [
                [persist.tile([128, 4, PAD], bf16, name=f"st{lh}{par}")
                 for par in range(2)]
                for lh in range(2)
            ]

            nc.sync.dma_start(out=e2_sb, in_=e2_d[:, :])
            nc.sync.dma_start(out=slab_sb, in_=slab_d[:, :])
            nc.vector.memset(vh[:, :, :, HD : HD + 1], 1.0)
            nc.vector.memset(ones1, 1.0)
            for lh in range(2):
                for par in range(2):
                    nc.gpsimd.memset(stripes[lh][par], 0.0)

            # ---------------- phase 1+2: loads + projections ----
            with ExitStack() as outer2:
                sc_ps = outer2.enter_context(
                    tc.tile_pool(name="sc_ps", bufs=5, space="PSUM")
                )
                qe_ps = sc_ps  # QE shares the scores PSUM slots (tag "sc")
                ctx_ps = None  # opened after mm_ps closes (PSUM bank budget)
                attT = outer2.enter_context(tc.tile_pool(name="attT", bufs=4))
                srl = outer2.enter_context(tc.tile_pool(name="srl", bufs=3))
                zp = outer2.enter_context(tc.tile_pool(name="zp", bufs=2))

                ident = persist.tile([128, 128], bf16, name="ident")
                from concourse.masks import make_identity

                make_identity(nc, ident)

                # short-lived input pools opened last (LIFO close order)
                tin_blk = ExitStack()
                tin = tin_blk.enter_context(tc.tile_pool(name="tin", bufs=1))
                mm_ps = tin_blk.enter_context(
                    tc.tile_pool(name="mm_ps", bufs=2, space="PSUM")
                )

                # qT and vT share one slot (vT loads after q-proj drains qT)
                qT = tin.tile([128, NDT, L], bf16, name="qT", tag="xqv")
                kT = tin.tile([128, NDT, L], bf16, name="kT")
                vT = tin.tile([128, NDT, L], bf16, name="vT", tag="xqv")
                wq_sb = tin.tile([128, NDT, DG], bf16, name="wq")
                wk_sb = tin.tile([128, NDT, DG], bf16, name="wk")
                wv_sb = tin.tile([128, NDT, DG], bf16, name="wv")

                def load_xT(dst, src_d):
                    src = bass.AP(
                        tensor=src_d,
                        offset=0,
                        ap=[[L, 128], [128 * L, NDT], [1, L]],
                    )
                    nc.sync.dma_start(out=dst, in_=src)

                def load_w(dst, src_d):
                    src = bass.AP(
                        tensor=src_d,
                        offset=0,
                        ap=[[DG, 128], [128 * DG, NDT], [1, DG]],
                    )
                    nc.sync.dma_start(out=dst, in_=src)

                # q + Wq first so qh projections (and QE) can start early;
                # whole-tensor batched loads (one trigger per tensor)
                load_w(wq_sb, wq_d)
                load_xT(qT, qT_d)
                load_w(wk_sb, wk_d)
                load_xT(kT, kT_d)
                load_w(wv_sb, wv_d)

                def proj_pair(w_sb, xT, dst, p):
                    for lh in range(2):
                        ps = mm_ps.tile([128, 512], f32, name="proj_ps", tag="mm")
                        lsl = slice(512 * lh, 512 * (lh + 1))
                        for dt in range(NDT):
                            nc.tensor.matmul(
                                ps,
                                w_sb[:, dt, 128 * p : 128 * (p + 1)],
                                xT[:, dt, lsl],
                                start=(dt == 0),
                                stop=(dt == NDT - 1),
                            )
                        nc.scalar.copy(dst[:, p, lsl], ps)

                def vh_tile(jt):
                    ps = mm_ps.tile([128, 512], f32, name="vh_ps", tag="mm")
                    jsl = slice(128 * jt, 128 * (jt + 1))
                    for dt in range(NDT):
                        nc.tensor.matmul(
                            ps,
                            vT[:, dt, jsl],
                            wv_sb[:, dt, :],
                            start=(dt == 0),
                            stop=(dt == NDT - 1),
                        )
                    # scatter 512 d' columns into per-head [64] slots with one
                    # strided copy (dst strides over the 65-wide head slots)
                    pv = ps[0:128, :]
                    ps3 = bass.AP(
                        tensor=pv.tensor,
                        offset=pv.offset,
                        ap=[list(pv.ap)[0], [HD, H_LOC], [1, HD]],
                    )
                    nc.scalar.copy(vh[:, jt, :, 0:HD], ps3)

                def qe_lt(h, lt):
                    """QE + masked padded stripe row-block lt for head h;
                    after the 4th block of an l-half, emit the stripe DMA."""
                    p, hl = divmod(h, 2)
                    rows = slice(64 * hl, 64 * (hl + 1))
                    tp = (64 * hl, 0)
                    lh, a = divmod(lt, 4)
                    big = stripes[lh][h % 2]
                    l0 = 128 * lt
                    lsl = slice(l0, l0 + 128)
                    stripe = big[:, a, :]
                    # QE only over the m-range that survives the tri mask
                    if lt <= 3:
                        n0 = l0 + 128
                        psm = qe_ps.tile([128, 512], f32, name="qe", tag="sc")
                        nc.tensor.matmul(
                            psm[:, 0:n0],
                            qhT[rows, p, lsl],
                            e2_sb[rows, 0:n0],
                            start=True,
                            stop=True,
                            tile_position=tp,
                        )
                        nc.vector.tensor_tensor(
                            stripe[:, 1 : 1 + n0],
                            psm[:, 0:n0],
                            slab_sb[:, 512 - l0 : 640],
                            mult,
                        )
                    else:
                        psm = qe_ps.tile([128, 512], f32, name="qe", tag="sc")
                        nc.tensor.matmul(
                            psm,
                            qhT[rows, p, lsl],
                            e2_sb[rows, 0:512],
                            start=True,
                            stop=True,
                            tile_position=tp,
                        )
                        n1 = l0 + 128 - 512
                        psm2 = qe_ps.tile([128, 512], f32, name="qe", tag="sc")
                        nc.tensor.matmul(
                            psm2[:, 0:n1],
                            qhT[rows, p, lsl],
                            e2_sb[rows, 512 : 512 + n1],
                            start=True,
                            stop=True,
                            tile_position=tp,
                        )
                        if lt == 4:
                            nc.vector.tensor_tensor(
                                stripe[:, 1:513],
                                psm,
                                slab_sb[:, 0:512],
                                mult,
                            )
                        else:
                            # m < 512 is fully below the diagonal: copy
                            nc.vector.tensor_copy(stripe[:, 1:513], psm)
                        nc.vector.tensor_tensor(
                            stripe[:, 513 : 1 + l0 + 128],
                            psm2[:, 0:n1],
                            slab_sb[:, 1024 - l0 : 640],
                            mult,
                        )
                    if a == 3:
                        # one DMA for the 4 padded stripes of this l-half
                        dst = bass.AP(
                            tensor=scratch[h],
                            offset=512 * lh * PAD,
                            ap=[[PAD, 128], [128 * PAD, 4], [1, PAD]],
                        )
                        nc.sync.dma_start(out=dst, in_=big)

                def srel_load(h, lh):
                    if lh == 0:
                        # low l-half: sliding 640-wide j-window per lt
                        # (window start 128*lt covers all nonzero blocks)
                        srel = srl.tile([128, 4, 640], bf16, name="srel")
                        src = bass.AP(
                            tensor=scratch[h],
                            offset=L,
                            ap=[[L, 128], [128 * L + 128, 4], [1, 640]],
                        )
                    else:
                        # high l-half: dense
                        srel = srl.tile([128, 4, L], bf16, name="srel")
                        src = bass.AP(
                            tensor=scratch[h],
                            offset=(512 * lh + 1) * L,
                            ap=[[L, 128], [128 * L, 4], [1, L]],
                        )
                    nc.sync.dma_start(out=srel, in_=src)
                    return srel

                def scores_tile(h, lh, jt, srel, att):
                    """scores^T + Srel^T + exp for one (l-half, j-tile)."""
                    p, hl = divmod(h, 2)
                    rows = slice(64 * hl, 64 * (hl + 1))
                    tp = (64 * hl, 0)
                    lsl = slice(512 * lh, 512 * (lh + 1))
                    jsl = slice(128 * jt, 128 * (jt + 1))
                    ps = sc_ps.tile([128, 512], f32, name="sc", tag="sc")
                    nzs = [
                        a for a in range(4)
                        if srel_block_nonzero(4 * lh + a, jt)
                    ]
                    # scores^T = kh qh^T for this (j-tile, l-half)
                    nc.tensor.matmul(
                        ps,
                        khT[rows, p, jsl],
                        qhT[rows, p, lsl],
                        start=True,
                        stop=(len(nzs) == 0),
                        tile_position=tp,
                    )
                    # += Srel^T via PE transpose-by-identity
                    for i, a in enumerate(nzs):
                        if lh == 0:
                            jr = slice(128 * (jt - a), 128 * (jt - a) + 128)
                            chunk = srel[:, a, jr]
                        else:
                            chunk = srel[:, a, jsl]
                        nc.tensor.matmul(
                            ps[:, 128 * a : 128 * a + 128],
                            chunk,
                            ident,
                            start=False,
                            stop=(i == len(nzs) - 1),
                        )
                    nc.scalar.activation(att[:, jt, :], ps, Exp, scale=0.125)

                def attnv_part(h, halves, lh, jt, cps):
                    nc.tensor.matmul(
                        cps[0 : HD + 1, :],
                        vh[:, jt, h, :],
                        halves[lh][:, jt, :],
                        start=(jt == 0),
                        stop=(jt == NLT - 1),
                    )

                def attnv_finish_a(cps0, zpack):
                    # stage the lh=0 Z row; the real finish happens in _b
                    nc.vector.tensor_copy(zpack[0:1, :], cps0[HD : HD + 1, :])

                def attnv_finish_b(h, cps_both, zpack):
                    p, hl = divmod(h, 2)
                    rows = slice(64 * hl, 64 * (hl + 1))
                    nc.vector.tensor_copy(
                        zpack[32:33, :], cps_both[1][HD : HD + 1, :]
                    )
                    # one fast approximate DVE reciprocal (~51 ULP, 5x faster
                    # than InstReciprocal) covers both Z rows (0 and 32; the
                    # rows between hold garbage and are never read)
                    from concourse.dve_ops import (
                        RECIP_APPROX_FAST_CONSTS,
                        RECIPROCAL_APPROX_FAST,
                    )

                    zinv = zp.tile([33, 512], f32, name="zinv")
                    nc.vector._custom_dve(
                        RECIPROCAL_APPROX_FAST,
                        out=zinv,
                        in0=zpack,
                        **RECIP_APPROX_FAST_CONSTS,
                    )
                    zinvb = [
                        zp.tile([1, 512], bf16, name="zinvb") for _ in range(2)
                    ]
                    for lh in range(2):
                        nc.vector.tensor_copy(
                            zinvb[lh], zinv[32 * lh : 32 * lh + 1, :]
                        )
                    for lh in range(2):
                        # broadcast across 64 partitions with a K=1 matmul
                        zb = sc_ps.tile([64, 512], f32, name="zb", tag="sc")
                        nc.tensor.matmul(
                            zb,
                            ones1,
                            zinvb[lh],
                            start=True,
                            stop=True,
                        )
                        zbs = zp.tile([64, 512], bf16, name="zbs")
                        nc.vector.tensor_copy(zbs, zb)
                        # normalize + pack into head-pair ctx^T (bf16)
                        nc.vector.tensor_tensor(
                            ctxp[rows, p, 512 * lh : 512 * (lh + 1)],
                            cps_both[lh][0:HD, :],
                            zbs,
                            mult,
                        )

                # ---- emission: projections first ----
                for p in range(NPAIR):
                    proj_pair(wq_sb, qT, qhT, p)
                # vT reuses qT's slot; its DMA fires once q-proj drains qT
                load_xT(vT, vT_d)
                for lt in range(8):
                    qe_lt(0, lt)
                for lt in range(8):
                    qe_lt(1, lt)
                for p in range(NPAIR):
                    proj_pair(wk_sb, kT, khT, p)
                for jt in range(NLT):
                    vh_tile(jt)
                tin_blk.close()
                ctx_ps = outer2.enter_context(
                    tc.tile_pool(name="ctx_ps", bufs=3, space="PSUM")
                )
                # wo lives in the space vacated by the input tiles; loaded
                # here (well before the output projection)
                wop = outer2.enter_context(tc.tile_pool(name="wop", bufs=1))
                wo_sb = wop.tile([128, NPAIR, D], bf16, name="wo")
                wo_src = bass.AP(
                    tensor=wo_d, offset=0, ap=[[D, 128], [128 * D, NPAIR], [1, D]]
                )
                nc.sync.dma_start(out=wo_sb, in_=wo_src)
                ost = outer2.enter_context(tc.tile_pool(name="ost", bufs=4))

                # ---- main pipeline: scores(h) / attnV(h-1) / QE(h+2)
                # interleaved at j-tile granularity so the in-order PE
                # always has a ready instruction ----
                pend = None
                fin = None  # deferred finish_b of head h-2
                srel_cur = srel_load(0, 0)
                for h in range(H_LOC):
                    att0 = attT.tile([128, NLT, 512], bf16, name="attnT")
                    att1 = attT.tile([128, NLT, 512], bf16, name="attnT")
                    # prefetch this head's high-half Srel during the low half
                    srel_nxt = srel_load(h, 1)
                    cps_prev = {}
                    zpack = zp.tile([33, 512], f32, name="zpack")
                    for jt in range(NLT):
                        if pend is not None:
                            hp, halves = pend
                            if jt == 0:
                                cps_prev[0] = ctx_ps.tile(
                                    [128, 512], f32, name="cps", tag="cps"
                                )
                            attnv_part(hp, halves, 0, jt, cps_prev[0])
                        scores_tile(h, 0, jt, srel_cur, att0)
                        if jt == 2 and fin is not None:
                            attnv_finish_b(*fin)
                            fin = None
                        if jt % 2 == 1 and h + 2 < H_LOC:
                            qe_lt(h + 2, jt // 2)
                    if pend is not None:
                        attnv_finish_a(cps_prev[0], zpack)
                    srel_cur = srel_nxt
                    if h + 1 < H_LOC:
                        # prefetch the next head's low half during this one
                        srel_nxt = srel_load(h + 1, 0)
                    for jt in range(NLT):
                        if pend is not None:
                            if jt == 0:
                                cps_prev[1] = ctx_ps.tile(
                                    [128, 512], f32, name="cps", tag="cps"
                                )
                            attnv_part(hp, halves, 1, jt, cps_prev[1])
                        scores_tile(h, 1, jt, srel_cur, att1)
                        if jt % 2 == 1 and h + 2 < H_LOC:
                            qe_lt(h + 2, 4 + jt // 2)
                    if pend is not None:
                        # defer the finish (PE broadcast + normalize) into the
                        # next head's stream so the head boundary never stalls
                        fin = (hp, cps_prev, zpack)
                    pend = (h, [att0, att1])
                    srel_cur = srel_nxt
                if fin is not None:
                    attnv_finish_b(*fin)
                    fin = None

                # ---- tail: attnV of the last head, interleaved with the
                # first half of the output projection ----
                def outproj_unit(lt, jh, o):
                    lsl = slice(128 * lt, 128 * (lt + 1))
                    jsl = slice(512 * jh, 512 * (jh + 1))
                    ps = sc_ps.tile([128, 512], f32, name="op", tag="sc")
                    for p in range(NPAIR):
                        nc.tensor.matmul(
                            ps,
                            ctxp[:, p, lsl],
                            wo_sb[:, p, jsl],
                            start=(p == 0),
                            stop=(p == NPAIR - 1),
                        )
                    nc.scalar.copy(o[:, jsl], ps)

                hp, halves = pend
                o_tiles = {}
                zpack = zp.tile([33, 512], f32, name="zpack")
                cps_last = {}
                cps_last[0] = ctx_ps.tile([128, 512], f32, name="cps", tag="cps")
                for jt in range(NLT):
                    attnv_part(hp, halves, 0, jt, cps_last[0])
                attnv_finish_a(cps_last[0], zpack)
                cps_last[1] = ctx_ps.tile([128, 512], f32, name="cps", tag="cps")
                for jt in range(NLT):
                    attnv_part(hp, halves, 1, jt, cps_last[1])
                attnv_finish_b(hp, cps_last, zpack)
                # out-proj over the lh=0 l-tiles follows the last normalize
                for jt in range(NLT):
                    lt, jh = jt // 2, jt % 2
                    if jh == 0:
                        o_tiles[lt] = ost.tile([128, D], bf16, name="o")
                    outproj_unit(lt, jh, o_tiles[lt])
                    if jh == 1:
                        lsl = slice(128 * lt, 128 * (lt + 1))
                        nc.sync.dma_start(out=out_d[lsl, :], in_=o_tiles[lt])
                for lt in range(4, NLT):
                    o = ost.tile([128, D], bf16, name="o")
                    for jh in range(2):
                        outproj_unit(lt, jh, o)
                    lsl = slice(128 * lt, 128 * (lt + 1))
                    nc.sync.dma_start(out=out_d[lsl, :], in_=o)

    nc.compile()
    return nc


TRACE = False
TRACE_KWARGS = {}
LAST_RESULT = None

_NC_CACHE = None


def _get_nc():
    global _NC_CACHE
    if _NC_CACHE is None:
        _NC_CACHE = _build_bass()
    return _NC_CACHE


def make_in_maps(k, v, q, E, Wk, Wv, Wq, Wo):
    """Host-side sharding: returns per-core input dicts."""
    eT = np.ascontiguousarray(E[MAX_SEQ - L :, :].T)  # [64, 1024]
    e2 = np.concatenate([eT, eT], axis=0).astype(BF16)  # [128, 1024]
    slab = (
        (np.arange(640)[None, :] - 512) <= np.arange(128)[:, None]
    ).astype(BF16)
    qkvT = {}
    for b in range(B):
        qkvT[b] = (
            np.ascontiguousarray(np.asarray(q[b]).T).astype(BF16),
            np.ascontiguousarray(np.asarray(k[b]).T).astype(BF16),
            np.ascontiguousarray(np.asarray(v[b]).T).astype(BF16),
        )
    in_maps = []
    for core in range(NCORES):
        b, hg = divmod(core, 2)
        csl = slice(DG * hg, DG * (hg + 1))
        qTb, kTb, vTb = qkvT[b]
        in_maps.append(
            {
                "qT": qTb,
                "kT": kTb,
                "vT": vTb,
                "wq": np.ascontiguousarray(Wq[:, csl]).astype(BF16),
                "wk": np.ascontiguousarray(Wk[:, csl]).astype(BF16),
                "wv": np.ascontiguousarray(Wv[:, csl]).astype(BF16),
                "wo": np.ascontiguousarray(Wo[DG * hg : DG * (hg + 1), :]).astype(BF16),
                "e2": e2,
                "slab": slab,
            }
        )
    return in_maps


def kernel(
    k,
    v,
    q,
    mask,
    E,
    Wk,
    bk,
    Wv,
    bv,
    Wq,
    bq,
    Wo,
    bo,
):
    k = np.asarray(k, np.float32)
    v = np.asarray(v, np.float32)
    q = np.asarray(q, np.float32)
    E = np.asarray(E, np.float32)
    Wk = np.asarray(Wk, np.float32)
    Wv = np.asarray(Wv, np.float32)
    Wq = np.asarray(Wq, np.float32)
    Wo = np.asarray(Wo, np.float32)
    mask = np.asarray(mask)
    assert bool(mask.all()), "kernel specialized for all-true mask"
    for bias in (bk, bv, bq):
        assert not np.any(np.asarray(bias)), "kernel specialized for zero qkv biases"
    bo = np.asarray(bo, np.float32)

    from concourse.bass_utils import run_bass_kernel_spmd

    nc = _get_nc()
    in_maps = make_in_maps(k, v, q, E, Wk, Wv, Wq, Wo)
    res = run_bass_kernel_spmd(
        nc, in_maps, core_ids=list(range(NCORES)), trace=TRACE, **TRACE_KWARGS
    )
    global LAST_RESULT
    LAST_RESULT = res
    out = np.zeros((B, L, D), np.float32)
    for core in range(NCORES):
        b = core // 2
        out[b] += np.asarray(res.results[core]["out"], np.float32)
    out += bo[None, None, :]
    return out
